# revision 4
# baseline (speedup 1.0000x reference)
"""
W8A8 quantized linear (dynamic per-token int8 activation quant + int8 weight,
fp32 dequant) on 8 Trainium2 NeuronCores.

Reference semantics (per token m, output channel n):
    absmax[m] = max_k |x[m,k]|            (fp32)
    scale[m]  = max(absmax[m]/127, 1e-8)
    q[m,k]    = round(x[m,k] / scale[m])  in [-127, 127]   (round-half-even)
    y[m,n]    = (sum_k q[m,k] * w[n,k]) * scale[m] * wscale[n]   -> fp16

Sharding: data-parallel over tokens (8192 tokens -> 1024/core); weight is
replicated. Host pre-transposes the weight to [K, N] and converts the int8
values to bf16 (exact: |w| <= 127 < 256).

Device kernel structure (engine assignment tuned from NTFF traces):
  warmup: ~76 dummy matmuls on zeroed SBUF keep the PE busy from t=0 so the
    HAM clock-gate is at 2.4 GHz when real matmuls arrive (and hide the
    quant-pipeline fill latency).
  phase A/B (per 128-token tile): x DMA on the VECTOR queue (own rings; not
    stuck behind the 4 MB weight DMA on sync); absmax via DVE reduce;
    scale/recip on DVE; quantize chunks on GPSIMD (x*inv + 1.5*2^23, fp32)
    then ACT (-1.5*2^23 -> bf16) [magic-number round-to-nearest-even];
    DMA-xbar-transpose per m-tile on the SYNC queue into qT[p, mt, kt, m].
  phase C: wt0+wt1 prefetched before phase A/B; per 512-wide N slice stream
    W^T and accumulate KT=32 matmuls per (m-tile, n-slice) into one PSUM
    bank; dequant on evacuation is ONE fused DVE op:
    scalar_tensor_tensor ot = (psum * scale[m]) * wscale[n] -> fp16,
    then y DMA on the SCALAR queue.
"""

import os
import numpy as np
import ml_dtypes
from contextlib import ExitStack

import concourse.bass as bass
import concourse.mybir as mybir
import concourse.tile as tile
from concourse import bacc

QMAX = 127.0
MAGIC = 1.5 * 2**23  # fp32 round-to-nearest-even trick for |v| < 2^22

F16 = mybir.dt.float16
BF16 = mybir.dt.bfloat16
F32 = mybir.dt.float32


def build_nc(M=1024, K=4096, N=4096, NSL=512, QCH=1024, WARM=76,
             do_quant=True, do_mm=True, reps=1):
    """One-core program; run SPMD on 8 cores with different token shards."""
    nc = bacc.Bacc()
    x = nc.declare_dram_parameter("x", [M, K], F16, isOutput=False)
    wT = nc.declare_dram_parameter("wT", [K, N], BF16, isOutput=False)
    wsb = nc.declare_dram_parameter("wsb", [128, N], F16, isOutput=False)
    y = nc.declare_dram_parameter("y", [M, N], F16, isOutput=True)

    MT, KT, NS = M // 128, K // 128, N // NSL
    QCH = min(QCH, K)
    wT3 = wT.rearrange("(kt p) n -> p kt n", p=128)

    with tile.TileContext(nc) as tc, ExitStack() as ctx:
      pers = ctx.enter_context(tc.tile_pool(name="pers", bufs=1))
      qpool = ctx.enter_context(tc.tile_pool(name="qt", bufs=1))
      xpool = ctx.enter_context(tc.tile_pool(name="xa", bufs=2))
      tpool = ctx.enter_context(tc.tile_pool(name="tmpq", bufs=2))
      qnat = ctx.enter_context(tc.tile_pool(name="qnat", bufs=2))
      wpool = ctx.enter_context(tc.tile_pool(name="wt", bufs=2))
      psum = ctx.enter_context(tc.tile_pool(name="psum", bufs=6, space="PSUM"))
      opool = ctx.enter_context(tc.tile_pool(name="out", bufs=3))
      for rep in range(reps):
        if rep > 0:
            tc.strict_bb_all_engine_barrier()

        # -- prefetch: first TWO weight slices (sync queue), wsb (scalar) --
        wt0 = wpool.tile([128, KT, NSL], BF16, tag="wt")
        wt1 = wpool.tile([128, KT, NSL], BF16, tag="wt")
        if do_mm:
            nc.sync.dma_start(wt0[:], wT3[:, :, 0:NSL])
            nc.sync.dma_start(wt1[:], wT3[:, :, NSL : 2 * NSL])
        wsb_sb = pers.tile([128, N], F16)
        nc.scalar.dma_start(wsb_sb[:], wsb[:])

        # -- PE warmup: dummy matmuls on zeroed SBUF from t~0 so HAM is at
        #    2.4 GHz (and stays there) when the first real matmul issues --
        if do_mm and WARM > 0:
            warm = pers.tile([128, NSL], BF16)
            nc.vector.memset(warm[:], 0.0)
            wps = psum.tile([128, NSL], F32, tag="pt")
            for _ in range(WARM):
                nc.tensor.matmul(wps[:], warm[:, 0:128], warm[:],
                                 start=True, stop=True)

        am = pers.tile([128, MT], F32)
        scales = pers.tile([128, MT], F32)
        invs = pers.tile([128, MT], F32)
        # m-tile-major so each m-tile's [KT,128] block is contiguous: one
        # xbar transpose per m-tile writes qT[p, mt, kt, m] = q[mt*128+m, kt*128+p]
        qT = qpool.tile([128, MT, KT, 128], BF16)

        if not do_quant:
            nc.vector.memset(scales[:], 1.0)
            if do_mm:
                nc.vector.memset(qT[:], 1.0)
        # ---- phase A/B: per m-tile absmax, scales, quantize, transpose ----
        for mt in range(MT if do_quant else 0):
            xa = xpool.tile([128, K], F16, tag="xa")
            nc.scalar.dma_start(xa[:], x[mt * 128 : (mt + 1) * 128, :])
            nc.vector.tensor_reduce(
                am[:, mt : mt + 1],
                xa[:],
                axis=mybir.AxisListType.X,
                op=mybir.AluOpType.max,
                apply_absolute_value=True,
            )
            # scale = max(absmax/127, 1e-8); inv = 1/scale
            nc.vector.tensor_scalar(
                scales[:, mt : mt + 1],
                am[:, mt : mt + 1],
                1.0 / QMAX,
                1e-8,
                mybir.AluOpType.mult,
                mybir.AluOpType.max,
            )
            nc.vector.reciprocal(invs[:, mt : mt + 1], scales[:, mt : mt + 1])

            qn = qnat.tile([128, K], BF16, tag="qn")
            for kc in range(K // QCH):
                sl = slice(kc * QCH, (kc + 1) * QCH)
                tmpq = tpool.tile([128, QCH], F32, tag="tmpq")
                # tmpq = x*inv + MAGIC  (fp32; rounds to integer at +MAGIC)
                nc.gpsimd.tensor_scalar(
                    tmpq[:],
                    xa[:, sl],
                    invs[:, mt : mt + 1],
                    MAGIC,
                    mybir.AluOpType.mult,
                    mybir.AluOpType.add,
                )
                # qn = tmpq - MAGIC  (exact; integer-valued, exact in bf16)
                nc.scalar.activation(
                    qn[:, sl],
                    tmpq[:],
                    mybir.ActivationFunctionType.Copy,
                    bias=-MAGIC,
                )

            # one xbar transpose for the whole m-tile: [128m, 4096k] ->
            # [128k-part, KT, 128m] (contiguous dst block)
            nc.sync.dma_start_transpose(qT[:, mt], qn[:, :])

        if not do_mm:
            ot0 = opool.tile([128, NSL], F16, tag="ot")
            nc.vector.memset(ot0[:], 0.0)
            nc.sync.dma_start(y[0:128, 0:NSL], ot0[:])
        # ---- phase C: matmul + fused dequant ----
        for ns in range(NS if do_mm else 0):
            nsl = slice(ns * NSL, (ns + 1) * NSL)
            if ns == 0:
                wt = wt0
            elif ns == 1:
                wt = wt1
            else:
                wt = wpool.tile([128, KT, NSL], BF16, tag="wt")
                nc.sync.dma_start(wt[:], wT3[:, :, nsl])
            for mt in range(MT):
                pt = psum.tile([128, NSL], F32, tag="pt")
                for kt in range(KT):
                    nc.tensor.matmul(
                        pt[:],
                        qT[:, mt, kt, :],
                        wt[:, kt, :],
                        start=(kt == 0),
                        stop=(kt == KT - 1),
                    )
                # ot = (psum * scale[m]) * wscale[n] -> fp16, one DVE op
                ot = opool.tile([128, NSL], F16, tag="ot")
                nc.vector.scalar_tensor_tensor(
                    ot[:],
                    pt[:],
                    scales[:, mt : mt + 1],
                    wsb_sb[:, nsl],
                    mybir.AluOpType.mult,
                    mybir.AluOpType.mult,
                )
                nc.scalar.dma_start(y[mt * 128 : (mt + 1) * 128, nsl], ot[:])

    nc.finalize()  # Bacc.compile(): reg alloc, wait-splitting, event sems
    return nc


def prep_inputs(x, weight, weight_scales, n_cores=8):
    """Host-side shard/layout prep. Returns (in_maps, out_assembler)."""
    B, S, D_in = x.shape
    D_out = weight.shape[0]
    M_total = B * S
    Mc = M_total // n_cores

    xf = np.ascontiguousarray(np.asarray(x).reshape(M_total, D_in))
    w = np.asarray(weight)
    wT = np.ascontiguousarray(w.T.astype(np.float32)).astype(ml_dtypes.bfloat16)
    ws = np.asarray(weight_scales).astype(np.float16)
    wsb = np.ascontiguousarray(np.broadcast_to(ws[None, :], (128, D_out)))

    in_maps = [
        {"x": xf[c * Mc : (c + 1) * Mc], "wT": wT, "wsb": wsb}
        for c in range(n_cores)
    ]

    def assemble(results):
        return np.concatenate(
            [np.asarray(results[c]["y"]) for c in range(n_cores)], axis=0
        ).reshape(B, S, D_out).astype(np.float16)

    return in_maps, assemble


def kernel(x, weight, weight_scales):
    from concourse.bass_utils import run_bass_kernel_spmd

    n_cores = 8
    B, S, D_in = x.shape
    D_out = weight.shape[0]
    Mc = (B * S) // n_cores

    nc = build_nc(M=Mc, K=D_in, N=D_out)
    in_maps, assemble = prep_inputs(x, weight, weight_scales, n_cores)
    res = run_bass_kernel_spmd(nc, in_maps, list(range(n_cores)))
    return assemble(res.results)


if __name__ == "__main__":
    np.random.seed(0)
    x = np.random.randn(4, 2048, 4096).astype(np.float16)
    w = np.random.randint(-127, 127, (4096, 4096)).astype(np.int8)
    ws = (np.random.rand(4096).astype(np.float32) * 0.01 + 1e-4).astype(np.float16)
    y = kernel(x, w, ws)
    print(y.shape, y.dtype)
